# revision 20
# baseline (speedup 1.0000x reference)
"""Trainium2 Bass kernel for the CustomLSTM problem.

Contract: kernel(**inputs) takes the FULL unsharded numpy inputs
(x [4096,16,512] f32, per-gate weights/biases) and returns the FULL
output h_last [4096, 1024] f32.

Strategy (data-parallel over 8 NeuronCores):
  - shard batch B=4096 -> 512 per core; replicate weights.
  - per core, per timestep t, compute fused gates in transposed layout
    gT [4H=4096, B=512]: one PSUM accumulation per 128-row gate tile.
  - on trn2 every matmul instruction costs ~213-235ns regardless of
    dtype (fp8 DoubleRow is LDWEIGHTS-bound at 256 cols, bf16 is
    stream-bound at 512 moving cols), so throughput == instruction
    count.  fp8 e4m3 with perf_mode=DoubleRow contracts K=256 per
    instruction, 2x bf16.  DR weight pair-tiles are stored contiguous
    ([2,128] blocks) -- a strided pair AP costs +22ns/matmul.
  - fp8 quantization noise injected at step t is damped ~0.5x per step
    by the forget gate; precision schedule (matmuls per gate tile):
      steps 0..11   fp8-DR everywhere (6)
      steps 12..14  x-part bf16 (exact), h-part fp8-DR (8)
      step  15      full bf16, weights streamed from DRAM (12)
    giving rel_err ~1.55e-2 (validated against a bit-exact numpy model
    of this pipeline).
  - sigmoid/tanh run on ScalarE straight out of PSUM with the per-gate
    bias applied via the activation instruction's per-partition bias
    (tanh(z) computed as 2*sigmoid(2z)-1 so only Sigmoid is used).
  - c stays fp32 in SBUF; gates are stored bf16; h is written fp8
    (bf16 when the consumer is the full-bf16 step); the DVE chain runs
    pair-wide ([128,1024] ops over two adjacent gate tiles) and DMAs
    are one-per-tile with host-side partition-major DRAM layouts.
"""

import numpy as np
import ml_dtypes

import concourse.bacc as bacc
import concourse.mybir as mybir
from concourse.tile import TileContext
from concourse.bass_utils import run_bass_kernel_spmd

F32 = mybir.dt.float32
BF16 = mybir.dt.bfloat16
FP8 = mybir.dt.float8e4
AF = mybir.ActivationFunctionType
DR = mybir.MatmulPerfMode.DoubleRow
MULT = mybir.AluOpType.mult
SUB = mybir.AluOpType.subtract

B, T, D, H = 4096, 16, 512, 1024
NCORES = 8
BL = B // NCORES          # batch per core
G = 4 * H                 # fused gate dim
KD = D // 128             # x contraction k-tiles
KH = H // 128             # h contraction k-tiles
KK = KD + KH              # total contraction k-tiles
NGT = G // 128            # gate tiles
S = 32.0                  # fp8 weight pre-scale (activation applies 1/S)

_BUILD_KWARGS = dict(n_mixed=3, n_full=1, bufs_g=3, bufs_x=2, bufs_h=2,
                     bufs_tmp=3, bufs_wu=2)


def build_lstm(nc, reps=1, n_mixed=4, n_full=1, bufs_g=3, bufs_x=2, bufs_h=2,
               bufs_tmp=3, bufs_wu=2, t_steps=None, nodep=False,
               tiny_act=False, no_dve=False):
    TS = t_steps if t_steps is not None else T   # timesteps actually computed
    SP0 = TS - n_mixed - n_full                  # first mixed step
    FULL0 = TS - n_full                          # first full-bf16 step
    NLATE = n_mixed + n_full
    # x, partition-major: row t*128+p holds [KD, BL] for timestep t
    xq_d = nc.declare_dram_parameter("xq", [max(SP0, 1) * 128, KD * BL], FP8,
                                     isOutput=False)
    x16_d = nc.declare_dram_parameter("x16", [max(NLATE, 1) * 128, KD * BL],
                                      BF16, isOutput=False)
    # fp8 DR weights: contiguous [2,128] pair blocks, idx = kk*NGT + gt
    wq_d = nc.declare_dram_parameter("wq", [128, (KD // 2) * NGT * 256], FP8,
                                     isOutput=False)
    uq_d = nc.declare_dram_parameter("uq", [128, (KH // 2) * NGT * 256], FP8,
                                     isOutput=False)
    # bf16 x-part weights for the mixed steps, pre-scaled by S
    w16_d = nc.declare_dram_parameter("w16", [128, KD * G], BF16,
                                      isOutput=False)
    # bf16 U weights (pre-scaled by S) for the full step, quad-major
    # chunks: row ht*128+p holds [KH, 512] with the U k-tile rows for gate
    # tiles {ht, 8+ht, 16+ht, 24+ht}; its x-part reuses the resident w16
    wu16_d = nc.declare_dram_parameter("wu16", [KH * 128, KH * 512], BF16,
                                       isOutput=False)
    b_d = nc.declare_dram_parameter("b", [128, 2 * NGT], F32, isOutput=False)
    out_d = nc.declare_dram_parameter("h_out", [H, BL], F32, isOutput=True)

    with TileContext(nc) as tc:
        with tc.tile_pool(name="const", bufs=1) as cpool, \
             tc.tile_pool(name="xp", bufs=bufs_x) as xpool, \
             tc.tile_pool(name="x16p", bufs=2) as x16pool, \
             tc.tile_pool(name="hqp", bufs=bufs_h) as hqpool, \
             tc.tile_pool(name="h16p", bufs=1) as h16pool, \
             tc.tile_pool(name="wup", bufs=bufs_wu) as wupool, \
             tc.tile_pool(name="gp", bufs=bufs_g) as gpool, \
             tc.tile_pool(name="tp", bufs=bufs_tmp) as tpool, \
             tc.tile_pool(name="ps", bufs=8, space="PSUM") as pspool:
            wq_sb = cpool.tile([128, (KD // 2) * NGT, 2, 128], FP8,
                               name="wq_sb")
            nc.sync.dma_start(out=wq_sb[:], in_=wq_d[:])
            uq_sb = cpool.tile([128, (KH // 2) * NGT, 2, 128], FP8,
                               name="uq_sb")
            w16_sb = cpool.tile([128, KD, G], BF16, name="w16_sb")

            def load_uq():
                nc.sync.dma_start(out=uq_sb[:], in_=uq_d[:])
                nc.sync.dma_start(out=w16_sb[:], in_=w16_d[:])
            if reps != 1:
                load_uq()
            b_sb = cpool.tile([128, 2 * NGT], F32, name="b_sb")
            nc.sync.dma_start(out=b_sb[:], in_=b_d[:])
            # c state, fp32; holds the fp32 output h after t = TS-1
            c_sb = cpool.tile([128, KH * BL], F32, name="c_sb")

            if no_dve:
                nc.vector.memset(c_sb[:], 0.0)
            hq_fake = h16_fake = None
            if nodep:
                # timing-diagnostic mode: h matmuls read a constant tile
                # (breaks the recurrence dependency; numerics wrong)
                hq_fake = cpool.tile([128, KH, BL], FP8, name="hq_fake")
                h16_fake = cpool.tile([128, KH, BL], BF16, name="h16_fake")
                for kh in range(KH):
                    nc.sync.dma_start(out=hq_fake[:, kh, :],
                                      in_=uq_d[:, 0:BL])
                    nc.sync.dma_start(out=h16_fake[:, kh, :],
                                      in_=wu16_d[0:128, 0:BL])

            def body(rep):
                h_prev = None
                for t in range(TS):
                    mixed = SP0 <= t < FULL0
                    full = t >= FULL0
                    nxt_full = (t + 1) >= FULL0 and (t + 1) < TS
                    if t < SP0:
                        x_t = xpool.tile([128, KD, BL], FP8,
                                         name=f"x_{rep}_{t}", tag="x")
                        nc.sync.dma_start(
                            out=x_t[:], in_=xq_d[t * 128:(t + 1) * 128, :])
                    else:
                        ti = t - SP0
                        x_t = x16pool.tile([128, KD, BL], BF16,
                                           name=f"x16_{rep}_{t}", tag="x16")
                        nc.sync.dma_start(
                            out=x_t[:], in_=x16_d[ti * 128:(ti + 1) * 128, :])
                    if t == 0 and reps == 1:
                        load_uq()   # first matmuls need only wq+x_0
                    h_new = None
                    if t < TS - 1:
                        if nxt_full:
                            h_new = h16pool.tile([128, KH, BL], BF16,
                                                 name=f"h16_{rep}_{t}",
                                                 tag="h16")
                        else:
                            h_new = hqpool.tile([128, KH, BL], FP8,
                                                name=f"h_{rep}_{t}", tag="h")
                    gates = None
                    for ht in range(KH):
                        wu = None
                        if full:
                            wu = wupool.tile([128, KH, 512], BF16,
                                             name=f"wu_{rep}_{t}_{ht}",
                                             tag="wu")
                            nc.sync.dma_start(
                                out=wu[:],
                                in_=wu16_d[ht * 128:(ht + 1) * 128, :])
                        if ht % 2 == 0:
                            # gate pair tile: [f_e f_o i_e i_o o_e o_o c_e c_o]
                            gates = gpool.tile([128, 8 * BL], BF16,
                                               name=f"gates_{rep}_{t}_{ht}",
                                               tag="g")
                        skip0 = 1 if t == 0 else 0  # skip f gate at t=0
                        pss = [(pspool.tile([128, BL], F32,
                                            name=f"ps_{rep}_{t}_{gi * KH + ht}",
                                            tag="ps")
                                if gi >= skip0 else None)
                               for gi in range(4)]
                        hq_in = (h16_fake if full else hq_fake) \
                            if nodep else h_prev
                        if mixed:
                            # batch all bf16 x-matmuls, then all fp8-DR
                            # h-matmuls: a bf16<->DR mode switch costs the
                            # PE ~0.4us, so switch twice per ht, not 8x
                            for gi in range(skip0, 4):
                                gt = gi * KH + ht
                                for kd in range(KD):
                                    nc.tensor.matmul(
                                        pss[gi][:],
                                        w16_sb[:, kd, gt * 128:gt * 128 + 128],
                                        x_t[:, kd, :],
                                        start=(kd == 0), stop=False)
                            for gi in range(skip0, 4):
                                gt = gi * KH + ht
                                for kk in range(KH // 2):
                                    nc.tensor.matmul(
                                        pss[gi][:],
                                        uq_sb[:, kk * NGT + gt, :, :],
                                        hq_in[:, 2 * kk:2 * kk + 2, :],
                                        start=False, stop=(kk == KH // 2 - 1),
                                        perf_mode=DR)
                        for gi in range(skip0, 4):
                            if mixed:
                                continue
                            gt = gi * KH + ht
                            if full:
                                # x-part from resident W*S; h-part from the
                                # streamed U*S slab (8.4MB vs 12.6MB)
                                for kd in range(KD):
                                    nc.tensor.matmul(
                                        pss[gi][:],
                                        w16_sb[:, kd, gt * 128:gt * 128 + 128],
                                        x_t[:, kd, :],
                                        start=(kd == 0), stop=False)
                                for kh in range(KH):
                                    nc.tensor.matmul(
                                        pss[gi][:],
                                        wu[:, kh, gi * 128:(gi + 1) * 128],
                                        hq_in[:, kh, :],
                                        start=False, stop=(kh == KH - 1))
                            else:
                                nmm = (KD // 2) + (KH // 2 if t > 0 else 0)
                                k = 0
                                for kk in range(KD // 2):
                                    nc.tensor.matmul(
                                        pss[gi][:],
                                        wq_sb[:, kk * NGT + gt, :, :],
                                        x_t[:, 2 * kk:2 * kk + 2, :],
                                        start=(k == 0), stop=(k == nmm - 1),
                                        perf_mode=DR)
                                    k += 1
                                if t > 0:
                                    for kk in range(KH // 2):
                                        nc.tensor.matmul(
                                            pss[gi][:],
                                            uq_sb[:, kk * NGT + gt, :, :],
                                            hq_in[:, 2 * kk:2 * kk + 2, :],
                                            start=False, stop=(k == nmm - 1),
                                            perf_mode=DR)
                                        k += 1
                        sc = 1.0 / S
                        for gi in range(4):
                            if t == 0 and gi == 0:
                                continue   # f gate unused at t=0 (c_prev=0)
                            gt = gi * KH + ht
                            gsl = (2 * gi + (ht % 2)) * BL
                            aw = 8 if tiny_act else BL
                            if gi == 3:
                                # tanh(z+b) = 2*sigmoid(2z+2b)-1; the affine
                                # part folds into the DVE consumers below
                                nc.scalar.activation(
                                    gates[:, gsl:gsl + aw], pss[gi][:, 0:aw],
                                    AF.Sigmoid,
                                    bias=b_sb[:, NGT + gt:NGT + gt + 1],
                                    scale=2.0 * sc)
                            else:
                                nc.scalar.activation(
                                    gates[:, gsl:gsl + aw], pss[gi][:, 0:aw],
                                    AF.Sigmoid, bias=b_sb[:, gt:gt + 1],
                                    scale=sc)
                        if ht % 2 == 0:
                            continue   # DVE chain runs pair-wide at odd ht
                        if no_dve:
                            continue
                        fp_ = gates[:, 0 * BL:2 * BL]
                        ip = gates[:, 2 * BL:4 * BL]
                        op_ = gates[:, 4 * BL:6 * BL]
                        sp = gates[:, 6 * BL:8 * BL]
                        csp = c_sb[:, (ht - 1) * BL:(ht + 1) * BL]
                        tmp = tpool.tile([128, 4 * BL], F32,
                                         name=f"tmp_{rep}_{t}_{ht}", tag="tmp")
                        t1p = tmp[:, 0:2 * BL]
                        t2p = tmp[:, 2 * BL:4 * BL]
                        # sp holds s = sigmoid(2z+2b); ch = 2s-1
                        # c_new = f*c + i*ch = f*c + (2*(i*s) - i)
                        if t == 0:
                            nc.vector.tensor_mul(t2p, ip, sp)
                            nc.vector.scalar_tensor_tensor(
                                csp, t2p, 2.0, ip, MULT, SUB)
                        else:
                            nc.vector.tensor_mul(t1p, fp_, csp)
                            nc.vector.tensor_mul(t2p, ip, sp)
                            nc.vector.scalar_tensor_tensor(
                                t2p, t2p, 2.0, ip, MULT, SUB)
                            nc.vector.tensor_add(csp, t1p, t2p)
                        # h = o*tanh(c); tanh(c) = 2*sigmoid(2c)-1
                        s2 = tpool.tile([128, 2 * BL], F32,
                                        name=f"s2_{rep}_{t}_{ht}",
                                        tag="s2", bufs=2)
                        nc.scalar.activation(s2[:], csp, AF.Sigmoid,
                                             scale=2.0)
                        nc.vector.tensor_mul(t1p, op_, s2[:])
                        hdst = (h_new[:, ht - 1:ht + 1, :]
                                if t < TS - 1 else csp)
                        nc.vector.scalar_tensor_tensor(
                            hdst, t1p, 2.0, op_, MULT, SUB)
                    h_prev = h_new
                for kh in range(KH):
                    nc.sync.dma_start(out=out_d[kh * 128:(kh + 1) * 128, :],
                                      in_=c_sb[:, kh * BL:(kh + 1) * BL])

            if reps == 1:
                body(0)
            else:
                with tc.For_i(0, reps, 1):
                    body(0)
    return nc


_BUILT = None


def _get_built():
    global _BUILT
    if _BUILT is None:
        nc = bacc.Bacc("TRN2", num_devices=NCORES)
        build_lstm(nc, **_BUILD_KWARGS)
        nc.compile()
        _BUILT = nc
    return _BUILT


def _pmajor(a, nk):
    """[nk*128, F] -> [128, nk*F] partition-major."""
    nkk, f = a.shape
    assert nkk == nk * 128
    return np.ascontiguousarray(
        a.reshape(nk, 128, f).transpose(1, 0, 2).reshape(128, nk * f))


def _drpairs(a):
    """[K, G] -> [128, (K//256)*NGT*256] contiguous DR pair blocks.

    Block idx = kk*NGT + gt holds [2, 128]: slot s, col c =
    a[kk*256 + s*128 + p, gt*128 + c].
    """
    K = a.shape[0]
    a5 = a.reshape(K // 256, 2, 128, NGT, 128)
    return np.ascontiguousarray(
        a5.transpose(2, 0, 3, 1, 4).reshape(128, (K // 256) * NGT * 256))


def _prep_inputs(x, wf, wi, wo, wc, uf, ui, uo, uc, bf, bi, bo, bc):
    E4 = mybir.dt.np(FP8)
    bf16 = ml_dtypes.bfloat16
    f32 = np.float32
    n_mixed = _BUILD_KWARGS["n_mixed"]
    n_full = _BUILD_KWARGS["n_full"]
    SP0 = T - n_mixed - n_full
    W = np.concatenate([wf, wi, wo, wc], axis=1).astype(f32)       # [D, 4H]
    U = np.concatenate([uf, ui, uo, uc], axis=1).astype(f32)       # [H, 4H]
    b = np.concatenate([bf, bi, bo, bc], axis=1).astype(f32)
    Wq = _drpairs((W * S).astype(E4))
    Uq = _drpairs((U * S).astype(E4))
    W16 = _pmajor((W * S).astype(bf16), KD)             # [128, KD*G]
    # quad-major bf16 U*S chunks for the full step, partition-major
    US = (U * S).astype(bf16)                                      # [H, G]
    slabs = []
    for ht in range(KH):
        cols = np.concatenate(
            [np.arange((gi * KH + ht) * 128, (gi * KH + ht) * 128 + 128)
             for gi in range(4)])
        slabs.append(_pmajor(US[:, cols], KH))          # [128, KH*512]
    WU16 = np.ascontiguousarray(np.vstack(slabs))       # [KH*128, KH*512]
    b_t = np.ascontiguousarray(
        np.concatenate([b.reshape(NGT, 128).T,
                        2.0 * b.reshape(NGT, 128).T], axis=1))  # [128, 2*NGT]
    # x [B, T, D] -> per-core partition-major [T*128, KD*BL]
    xt = np.ascontiguousarray(np.transpose(x, (1, 2, 0))).astype(f32)
    xq = xt[:SP0].astype(E4)
    x16 = xt[SP0:].astype(bf16)

    def _xmajor(a):
        # [Tn, D, BLc] -> [Tn*128, KD*BLc]: row t*128+p holds [kd, b]
        tn, d, blc = a.shape
        return np.ascontiguousarray(
            a.reshape(tn, KD, 128, blc).transpose(0, 2, 1, 3).reshape(
                tn * 128, KD * blc))

    in_maps = []
    for c in range(NCORES):
        sl = slice(c * BL, (c + 1) * BL)
        in_maps.append({
            "xq": _xmajor(xq[:, :, sl]),
            "x16": _xmajor(x16[:, :, sl]),
            "wq": Wq, "uq": Uq, "w16": W16, "wu16": WU16, "b": b_t,
        })
    return in_maps


def kernel(x, wf, wi, wo, wc, uf, ui, uo, uc, bf, bi, bo, bc):
    nc = _get_built()
    in_maps = _prep_inputs(x, wf, wi, wo, wc, uf, ui, uo, uc, bf, bi, bo, bc)
    res = run_bass_kernel_spmd(nc, in_maps, list(range(NCORES)))
    out = np.empty((B, H), np.float32)
    for c in range(NCORES):
        out[c * BL:(c + 1) * BL, :] = res.results[c]["h_out"].T
    return out
